# revision 1
# baseline (speedup 1.0000x reference)
"""Trainium2 Bass kernel for nn_AttenCross (sparse_attention).

reference:
    scores = einsum('bqd,bkd->bqk', Q, D) / sqrt(H)
    scores = where(doc_mask==0, -9999, scores)
    attn   = softmax(scores, -1)
    out    = sum over k of (attn * sim), then sum over q -> (B, 1)

Strategy (8 cores, data-parallel over batch, 2 batches/core).
Host-side prep (sharding/layout/encoding only, exact for any inputs):
slice per core; transpose Q and D to [H, L] layout (the PE contracts
over partitions); apply the doc mask by zeroing masked doc rows of D
and masked sim columns; pass the per-batch masked count.  With D rows
zeroed, masked scores are exactly 0 so exp gives exactly 1: subtracting
the masked count from the exp row-sum reproduces the exact softmax
denominator, and masked sim columns are zero so they add nothing to the
numerator.  (No row-max subtraction: scores ~ N(0,1); softmax is
shift-invariant.)

Device, per batch:
  - round Q^T/D^T to fp32r (PE full-rate fp32 mode; inputs are rounded
    to 12-bit mantissa, exact multiply, fp32 accumulate).
  - per q-tile (128 queries): fp32r QK^T matmuls into PSUM; ACT computes
    E = exp(scale*psum) into SBUF with fused accum_out row-sums (den).
  - DVE: fused multiply P = E * sim (fp32r out), den fixup, reciprocal.
  - PE: column-sum matmuls with 1/den as the stationary operand (only
    column 0 of a [128,128] fp32r tile nonzero) accumulate
    sum_q P[q,k]/den_q into one [128,512] PSUM bank across all q-tiles
    and segments; epilogue reduces that bank to the scalar output.
Output per core: [2, 1]; host stacks to [16, 1] fp32.
"""

import numpy as np

import concourse.bacc as bacc
import concourse.tile as tile
import concourse.mybir as mybir
from concourse.bass_utils import run_bass_kernel_spmd

B, QL, DL, H = 16, 1024, 4096, 128
NCORES = 8
BPC = B // NCORES  # batches per core
QT_N = QL // 128  # 8 q-tiles per batch
SEG = 512
NSEG = DL // SEG  # 8
CH = 1024
NCH = DL // CH  # 4
SCALE = 1.0 / float(np.sqrt(H))

f32 = mybir.dt.float32
f32r = mybir.dt.float32r

_CACHED = {}


def _build():
    nc = bacc.Bacc("TRN2", target_bir_lowering=False, debug=False)

    qtd = nc.dram_tensor("qt", [BPC, H, QL], f32, kind="ExternalInput")
    dtd = nc.dram_tensor("dt", [BPC, H, DL], f32, kind="ExternalInput")
    sd = nc.dram_tensor("s", [BPC, QL, DL], f32, kind="ExternalInput")
    cntd = nc.dram_tensor("cnt", [BPC, 1], f32, kind="ExternalInput")
    outd = nc.dram_tensor("o", [BPC, 1], f32, kind="ExternalOutput")

    with tile.TileContext(nc) as tc:
        with (
            tc.tile_pool(name="const", bufs=1) as const,
            tc.tile_pool(name="raw", bufs=1) as raw,
            tc.tile_pool(name="b2", bufs=2) as b2,
            tc.tile_pool(name="dtp", bufs=2) as dtp,
            tc.tile_pool(name="simp", bufs=3) as simp,
            tc.tile_pool(name="ep", bufs=2) as ep,
            tc.tile_pool(name="pp", bufs=2) as pp,
            tc.tile_pool(name="small", bufs=4) as small,
            tc.tile_pool(name="bsm", bufs=2) as bsm,
            tc.tile_pool(name="pscore", bufs=3, space="PSUM") as pscore,
            tc.tile_pool(name="pacc", bufs=1, space="PSUM") as pacc,
            tc.tile_pool(name="ptp", bufs=1, space="PSUM") as ptp,
        ):
            ones128 = const.tile([128, 1], f32, tag="ones128")
            nc.vector.memset(ones128, 1.0)
            z128 = const.tile([128, 128], f32, tag="z128")
            nc.vector.memset(z128, 0.0)
            r128a = const.tile([128, 128], f32r, tag="r128a")
            nc.vector.tensor_copy(r128a, z128)
            r128b = const.tile([128, 128], f32r, tag="r128b")
            nc.vector.tensor_copy(r128b, z128)

            # HAM warm-up: full-array bf16 matmuls during the DMA ramp so the
            # PE clock gate is at 2.4 GHz before the first scores matmul
            w16 = const.tile([128, SEG], mybir.dt.bfloat16, tag="w16")
            nc.vector.memset(w16, 0.001)
            for i in range(16):
                pd = ptp.tile([128, SEG], f32, tag="tp", name=f"warm{i}")
                nc.tensor.matmul(pd, w16[:, :128], w16, start=True, stop=True)

            for b in range(BPC):
                # ---- per-batch loads + fp32r rounding (dt split for ramp) ----
                qtraw = raw.tile([128, QL], f32, tag="qtraw")
                nc.sync.dma_start(qtraw, qtd.ap()[b])
                qt = b2.tile([128, QL], f32r, tag="qt")
                nc.vector.tensor_copy(qt, qtraw)
                dtraw = raw.tile([128, DL], f32, tag="dtraw")
                dt = dtp.tile([128, DL], f32r, tag="dt")
                half = DL // 2
                for hh in range(2):
                    sl = slice(hh * half, (hh + 1) * half)
                    nc.sync.dma_start(dtraw[:, sl], dtd.ap()[b][:, sl])
                    nc.vector.tensor_copy(dt[:, sl], dtraw[:, sl])

                # den correction: crep[q] = masked count, replicated via
                # partition-broadcast DMA
                crep = bsm.tile([128, 1], f32, tag="crep")
                cnt_ap = cntd.ap()[b : b + 1, :]
                import concourse.bass as _bass
                cnt_bcast = _bass.AP(
                    tensor=cnt_ap.tensor,
                    offset=cnt_ap.offset,
                    ap=[[0, 128], [1, 1]],
                )
                nc.sync.dma_start(crep, cnt_bcast)

                # column-sum accumulator: row 0 collects sum_q P[q,k]/den_q
                acc = pacc.tile([128, SEG], f32, tag="acc")

                # ---- q-tiles ----
                for t in range(QT_N):
                    sim_h = []
                    e_h = []
                    p_h = []
                    for hh in range(2):
                        s_t = simp.tile(
                            [128, half], f32, tag=f"sim{hh}", name=f"sim{hh}_{b}_{t}"
                        )
                        nc.sync.dma_start(
                            s_t,
                            sd.ap()[
                                b, t * 128 : (t + 1) * 128,
                                hh * half : (hh + 1) * half,
                            ],
                        )
                        sim_h.append(s_t)
                        e_h.append(
                            ep.tile([128, half], f32, tag=f"e{hh}", name=f"e{hh}_{b}_{t}")
                        )
                        p_h.append(
                            pp.tile([128, half], f32r, tag=f"p{hh}", name=f"p{hh}_{b}_{t}")
                        )
                    den4 = small.tile([128, NCH], f32, tag="den4")
                    for c in range(NCH):
                        psc = pscore.tile([128, CH], f32, tag="sc")
                        for hh in range(CH // SEG):
                            off = c * CH + hh * SEG
                            nc.tensor.matmul(
                                psc[:, hh * SEG : (hh + 1) * SEG],
                                qt[:, t * 128 : (t + 1) * 128],
                                dt[:, off : off + SEG],
                                start=True,
                                stop=True,
                            )
                        eh = e_h[c // 2]
                        eo = (c % 2) * CH
                        nc.scalar.activation(
                            out=eh[:, eo : eo + CH],
                            in_=psc,
                            func=mybir.ActivationFunctionType.Exp,
                            scale=SCALE,
                            accum_out=den4[:, c : c + 1],
                        )

                    den = small.tile([128, 1], f32, tag="den")
                    nc.vector.reduce_sum(den, den4, axis=mybir.AxisListType.X)
                    dent = small.tile([128, 1], f32, tag="dent")
                    nc.vector.tensor_scalar(
                        dent, den, crep, None, mybir.AluOpType.subtract
                    )
                    r128 = r128a if t % 2 == 0 else r128b
                    # same numerics as fp32 reciprocal + f32r rounding copy,
                    # one DVE op shorter on the critical chain
                    with nc.allow_low_precision(
                        reason="1/den rounded to f32r for the PE colsum, "
                        "identical to the previous copy-based rounding"
                    ):
                        nc.vector.reciprocal(r128[:, 0:1], dent)

                    for qq in range(4):
                        hh, lo = qq // 2, (qq % 2) * CH
                        nc.vector.tensor_tensor(
                            p_h[hh][:, lo : lo + CH],
                            e_h[hh][:, lo : lo + CH],
                            sim_h[hh][:, lo : lo + CH],
                            mybir.AluOpType.mult,
                        )

                    for j in range(NSEG):
                        nc.tensor.matmul(
                            acc,
                            r128,
                            p_h[j // 4][:, (j % 4) * SEG : (j % 4 + 1) * SEG],
                            start=(t == 0 and j == 0),
                            stop=(t == QT_N - 1 and j == NSEG - 1),
                            skip_group_check=True,
                        )

                # ---- batch epilogue ----
                red128 = bsm.tile([128, 1], f32, tag="red128")
                nc.vector.reduce_sum(red128, acc, axis=mybir.AxisListType.X)
                ps_o = ptp.tile([1, 1], f32, tag="tp")
                nc.tensor.matmul(ps_o, red128, ones128, start=True, stop=True)
                out_sb = bsm.tile([1, 1], f32, tag="out_sb")
                nc.vector.tensor_copy(out_sb, ps_o)
                nc.sync.dma_start(outd.ap()[b : b + 1, :], out_sb)

    nc.compile()
    return nc


def kernel(**inputs: np.ndarray) -> np.ndarray:
    if "nc" not in _CACHED:
        _CACHED["nc"] = _build()
    nc = _CACHED["nc"]

    q = np.asarray(inputs["query_input"], dtype=np.float32)
    d = np.asarray(inputs["doc_input"], dtype=np.float32)
    s = np.asarray(inputs["sim_matrix"], dtype=np.float32)
    dm = (np.asarray(inputs["doc_mask"]) != 0).astype(np.float32)  # [B, DL]

    qt = np.ascontiguousarray(np.swapaxes(q, 1, 2))  # [B, H, QL]
    dt = np.ascontiguousarray(np.swapaxes(d * dm[:, :, None], 1, 2))  # [B, H, DL]
    sm = np.ascontiguousarray(s * dm[:, None, :])  # [B, QL, DL]
    cnt = (DL - dm.sum(axis=1, keepdims=True)).astype(np.float32)  # [B, 1]

    in_maps = []
    for c in range(NCORES):
        lo, hi = c * BPC, (c + 1) * BPC
        in_maps.append(
            {
                "qt": qt[lo:hi],
                "dt": dt[lo:hi],
                "s": sm[lo:hi],
                "cnt": cnt[lo:hi],
            }
        )

    res = None
    for attempt in range(3):
        try:
            res = run_bass_kernel_spmd(nc, in_maps, core_ids=list(range(NCORES)))
            break
        except Exception:
            if attempt == 2:
                raise
    out = np.concatenate([res.results[c]["o"] for c in range(NCORES)], axis=0)
    return out.astype(np.float32)



# revision 8
# speedup vs baseline: 1.7669x; 1.7669x over previous
"""Trainium2 Bass kernel for nn_AttenCross (sparse_attention).

reference:
    scores = einsum('bqd,bkd->bqk', Q, D) / sqrt(H)
    scores = where(doc_mask==0, -9999, scores)
    attn   = softmax(scores, -1)
    out    = sum over k of (attn * sim), then sum over q -> (B, 1)

Strategy (8 cores, data-parallel over batch, 2 batches/core), v2:

Host-side prep (sharding/layout/encoding only, exact for any inputs):
~50% of doc positions are masked (doc_mask ~ Bernoulli(0.5)), and masked
columns contribute exactly nothing once handled by counting, so the doc
axis is COMPACTED host-side: keep only unmasked doc columns (of D and
sim), zero-pad to K_pad = ceil(max_b keff[b]/128)*128.  A padded column
of D is all-zero => its score is exactly 0 => exp is exactly 1, so
subtracting the pad count from the exp row-sum reproduces the exact
softmax denominator; padded sim columns are zero so they add nothing to
the numerator.  (No row-max subtraction: scores ~ N(0,1); softmax is
shift-invariant.)  All tensors are converted to fp16 host-side: PE runs
fp16 at full rate (1 col/cycle vs ~2.2 for f32r), DMA bytes halve, and
DVE 16-bit ops run in 2x/4x perf modes; fp16's 10-bit mantissa keeps the
overall rel-err ~1e-3.

Device, per batch (per q-tile of 128 queries):
  - PE: QK^T fp16 matmuls into PSUM chunks (1536 + K_pad-1536 wide).
  - ACT: E = exp(scale*psum) -> fp16 SBUF, one ACTIVATE per chunk, with
    accum_out producing the per-chunk row-sums (den parts) for free-ish
    (the DVE alternatives -- tensor_scalar-with-accum aka
    TENSOR_SCALAR_CACHE_REDUCE, and tensor_reduce -- both run at 1x
    ~2.5us per q-tile on HW; ACT's ACTIVATION_READ_ACCUMULATOR is 311ns).
  - DVE: den-parts sum + cnt subtract + reciprocal -> w into column 0 of
    an alternating fp16 [128,128] tile; P = E * sim fp16 (2x mode).
  - PE: column-sum matmuls with w as the stationary operand accumulate
    sum_q P[q,k]/den_q into a per-batch [128,512] PSUM bank; row 0 of
    that bank reduced (DVE) to the scalar batch output at the end.
Output per core: [1, BPC] fp32; host stacks to [16, 1] fp32.
"""

import math

import numpy as np

import concourse.bacc as bacc
import concourse.tile as tile
import concourse.mybir as mybir
from concourse.bass_utils import run_bass_kernel_spmd

B, QL, DL, H = 16, 1024, 4096, 128
NCORES = 8
BPC = B // NCORES  # batches per core
QT_N = QL // 128  # 8 q-tiles per batch
SCALE = 1.0 / float(np.sqrt(H))

f32 = mybir.dt.float32
f16 = mybir.dt.float16

_CACHED = {}


def _plan_chunks(k_pad):
    """Split the doc axis into PSUM-resident chunks.

    Expected path (k_pad <= 2560): (1536, rest) -> 3+2 PSUM banks, one
    ACTIVATE per chunk.  Generic fallback for larger k_pad: 1024-wide
    chunks cycling three tags (2+2+2 banks)."""
    if k_pad <= 1536:
        return [(0, k_pad, "A")]
    if k_pad <= 2560:
        return [(0, 1536, "A"), (1536, k_pad, "B")]
    chunks = []
    off = 0
    i = 0
    while off < k_pad:
        w = min(1024, k_pad - off)
        chunks.append((off, off + w, "ABC"[i % 3]))
        off += w
        i += 1
    return chunks


def _build(k_pad):
    chunks = _plan_chunks(k_pad)

    nc = bacc.Bacc("TRN2", target_bir_lowering=False, debug=False)

    qtd = nc.dram_tensor("qt", [BPC, H, QL], f16, kind="ExternalInput")
    dtd = nc.dram_tensor("dt", [BPC, H, k_pad], f16, kind="ExternalInput")
    sd = nc.dram_tensor("s", [BPC, QL, k_pad], f16, kind="ExternalInput")
    ckd = nc.dram_tensor("ck", [BPC, 1], f32, kind="ExternalInput")
    outd = nc.dram_tensor("o", [1, BPC], f32, kind="ExternalOutput")

    with tile.TileContext(nc) as tc:
        with (
            tc.tile_pool(name="const", bufs=1) as const,
            tc.tile_pool(name="qtp", bufs=2) as qtp,
            tc.tile_pool(name="dtp", bufs=2) as dtp,
            tc.tile_pool(name="simp", bufs=4) as simp,
            tc.tile_pool(name="ep", bufs=2) as ep,
            tc.tile_pool(name="pp", bufs=2) as pp,
            tc.tile_pool(name="small", bufs=4) as small,
            tc.tile_pool(name="bsm", bufs=2) as bsm,
            tc.tile_pool(name="outp", bufs=1) as outp,
            tc.tile_pool(name="ps", bufs=1, space="PSUM") as psp,
            tc.tile_pool(name="pacc", bufs=2, space="PSUM") as pacc,
            tc.tile_pool(name="pwarm", bufs=1, space="PSUM") as pwarm,
        ):
            z128 = const.tile([128, 128], f32, tag="z128")
            nc.vector.memset(z128, 0.0)
            # stationary w-tiles for the column-sum matmuls: col 0 = 1/den,
            # cols 1..127 stay zero forever
            r128a = const.tile([128, 128], f16, tag="r128a")
            nc.vector.memset(r128a, 0.0)
            r128b = const.tile([128, 128], f16, tag="r128b")
            nc.vector.memset(r128b, 0.0)

            outsb = outp.tile([1, BPC], f32, tag="outsb")

            # HAM warm-up: full-array fp16 matmuls during the DMA ramp so the
            # PE clock gate is at 2.4 GHz before the first scores matmul
            w16 = const.tile([128, 512], f16, tag="w16")
            nc.vector.memset(w16, 0.001)
            warm = pwarm.tile([128, 512], f32, tag="warm")
            for i in range(16):
                nc.tensor.matmul(
                    warm, w16[:, :128], w16, start=True, stop=True,
                    skip_group_check=True,
                )

            import concourse.bass as _bass

            for b in range(BPC):
                qt = qtp.tile([128, QL], f16, tag="qt", name=f"qt{b}")
                nc.sync.dma_start(qt, qtd.ap()[b])
                dt = dtp.tile([128, k_pad], f16, tag="dt", name=f"dt{b}")
                for s0 in range(0, k_pad, 1024):
                    s1 = min(s0 + 1024, k_pad)
                    nc.sync.dma_start(dt[:, s0:s1], dtd.ap()[b][:, s0:s1])

                # pad count replicated to all 128 partitions via
                # partition-broadcast DMA
                cntk = bsm.tile([128, 1], f32, tag="cntk", name=f"cntk{b}")
                ck_ap = ckd.ap()[b : b + 1, :]
                ck_bcast = _bass.AP(
                    tensor=ck_ap.tensor,
                    offset=ck_ap.offset,
                    ap=[[0, 128], [1, 1]],
                )
                nc.sync.dma_start(cntk, ck_bcast)

                acc = pacc.tile([128, 512], f32, tag="acc", name=f"acc{b}")

                ncol = (k_pad + 511) // 512  # column-sum segments
                nchunks = len(chunks)
                for t in range(QT_N):
                    sim = simp.tile(
                        [128, k_pad], f16, tag="sim", name=f"sim{b}_{t}"
                    )
                    # split at a 2048B (1024-elem) boundary: whole-packet
                    # per-partition runs DMA faster than a 4352B run
                    for s0 in range(0, k_pad, 1024):
                        s1 = min(s0 + 1024, k_pad)
                        nc.sync.dma_start(
                            sim[:, s0:s1],
                            sd.ap()[b, t * 128 : (t + 1) * 128, s0:s1],
                        )
                    e_t = ep.tile([128, k_pad], f16, tag="E", name=f"e{b}_{t}")
                    den2 = small.tile(
                        [128, nchunks], f32, tag="den2", name=f"den2_{b}_{t}"
                    )

                    for ci, (lo, hi, tag) in enumerate(chunks):
                        psc = psp.tile(
                            [128, hi - lo], f32, tag=tag, name=f"psc{tag}"
                        )
                        for s0 in range(0, hi - lo, 512):
                            s1 = min(s0 + 512, hi - lo)
                            nc.tensor.matmul(
                                psc[:, s0:s1],
                                qt[:, t * 128 : (t + 1) * 128],
                                dt[:, lo + s0 : lo + s1],
                                start=True,
                                stop=True,
                            )
                        nc.scalar.activation(
                            out=e_t[:, lo:hi],
                            in_=psc,
                            func=mybir.ActivationFunctionType.Exp,
                            scale=SCALE,
                            accum_out=den2[:, ci : ci + 1],
                        )

                    den = small.tile([128, 1], f32, tag="den", name="den")
                    nc.vector.reduce_sum(den, den2, axis=mybir.AxisListType.X)
                    dent = small.tile([128, 1], f32, tag="dent", name="dent")
                    nc.vector.tensor_scalar(
                        dent, den, cntk, None, mybir.AluOpType.subtract
                    )
                    r128 = r128a if t % 2 == 0 else r128b
                    with nc.allow_low_precision(
                        reason="1/den in fp16 (11-bit mantissa) feeds the PE "
                        "column-sum; ~5e-4 relative, inside the error budget"
                    ):
                        nc.vector.reciprocal(r128[:, 0:1], dent)

                    p_t = pp.tile([128, k_pad], f16, tag="P", name=f"p{b}_{t}")
                    nc.vector.tensor_tensor(
                        p_t, e_t, sim, mybir.AluOpType.mult
                    )

                    for j in range(ncol):
                        s0 = j * 512
                        s1 = min(s0 + 512, k_pad)
                        nc.tensor.matmul(
                            acc[:, : s1 - s0],
                            r128,
                            p_t[:, s0:s1],
                            start=(t == 0 and j == 0),
                            stop=(t == QT_N - 1 and j == ncol - 1),
                            skip_group_check=True,
                        )

                # batch epilogue: row 0 of acc holds sum_q P[q,k]/den_q
                nc.vector.reduce_sum(
                    outsb[0:1, b : b + 1], acc[0:1, :], axis=mybir.AxisListType.X
                )

            nc.sync.dma_start(outd.ap()[:, :], outsb)

    nc.compile()
    return nc


def kernel(**inputs: np.ndarray) -> np.ndarray:
    q = np.asarray(inputs["query_input"], dtype=np.float32)
    d = np.asarray(inputs["doc_input"], dtype=np.float32)
    s = np.asarray(inputs["sim_matrix"], dtype=np.float32)
    dm = np.asarray(inputs["doc_mask"]) != 0  # [B, DL]

    keff = dm.sum(axis=1).astype(np.int64)  # [B]
    k_pad = int(min(DL, max(128, math.ceil(int(keff.max()) / 128) * 128)))

    if k_pad not in _CACHED:
        _CACHED[k_pad] = _build(k_pad)
    nc = _CACHED[k_pad]

    qt = np.ascontiguousarray(np.swapaxes(q, 1, 2)).astype(np.float16)
    dtc = np.zeros((B, H, k_pad), dtype=np.float16)
    simc = np.zeros((B, QL, k_pad), dtype=np.float16)
    for b in range(B):
        idx = np.flatnonzero(dm[b])
        ke = idx.size
        dtc[b, :, :ke] = d[b, idx, :].T
        simc[b, :, :ke] = s[b][:, idx]
    ck = (k_pad - keff).astype(np.float32).reshape(B, 1)

    in_maps = []
    for c in range(NCORES):
        lo, hi = c * BPC, (c + 1) * BPC
        in_maps.append(
            {
                "qt": qt[lo:hi],
                "dt": dtc[lo:hi],
                "s": simc[lo:hi],
                "ck": ck[lo:hi],
            }
        )

    res = None
    for attempt in range(3):
        try:
            res = run_bass_kernel_spmd(nc, in_maps, core_ids=list(range(NCORES)))
            break
        except Exception:
            if attempt == 2:
                raise
    out = np.concatenate(
        [res.results[c]["o"].reshape(BPC) for c in range(NCORES)], axis=0
    )
    return out.reshape(B, 1).astype(np.float32)
